# revision 12
# baseline (speedup 1.0000x reference)
"""Trainium2 Bass kernel for entmax-1.5 over rows of a masked [8192, 4096] matrix.

Algorithm (sort-free, validated against the jax reference in float32):
  p_i = relu(z_i - tau)^2 per row, tau s.t. sum_i p_i = 1, z = masked_scores/2.
  Device works in "half-units" t = (0.5*s + 15)*mask (masked -> 0, ~14 below
  any feasible threshold), so p = relu(t - a)^2 with a = rowmax(t) + tau and
  no rescaling anywhere:
    1. top-8 per row (DVE InstMax) -> closed-form entmax threshold of the
       top-8 subset (reference recursion on 8 sorted values, batched across
       tiles, gather-free support selection). Guaranteed lower bound of tau
       (support size is ~21, max 46).
    2. 3 Newton refinements on the full row:  u = relu(t - a) via ACT with
       per-partition bias, accum_out -> h = sum u;  F = sum u^2 measured on
       iters 0,1 (ACT Square+accum / DVE square+reduce, split to balance
       engines) and propagated on iter 2 via the trapezoid identity
       F' = F - (h_prev + h) * delta  (f is piecewise quadratic in tau);
       a += (F - 1) / (2 h).
    3. p = relu(t - a)^2  (ACT Relu;Square on half the tiles, DVE on rest).

Sharding: pure data parallelism — 8192 rows = 1024 rows x 8 cores; per core
8 tiles of [128 partitions x 4096], processed as 2 groups of 4 whose phases
interleave (group 2 loads/masks overlap group 1 compute).

Hardware constraints worked around: pseudo-DMA / ACT / TT instructions accept
very few sync-waits, so scores+mask are packed into one u8 DMA per tile (the
mask-fold waits on a single queue semaphore), packed tiles get dedicated SBUF
slots, and bacc's event-semaphore pass splits remaining multi-waits.
tensor_tensor_reduce crashes this runtime and is not used. Group-2 mask-folds
run on GPSIMD to keep DVE free.

Self-contained: hardcodes scores[8192,4096] f32 + mask[8192,4096] bool.
"""

import sys

import numpy as np

sys.path.insert(0, "/opt/trn_rl_repo")

N_ROWS = 8192
N_COLS = 4096
N_CORES = 8
P = 128
ROWS_PER_CORE = N_ROWS // N_CORES          # 1024
NT = ROWS_PER_CORE // P                    # 8 tiles per core
N_ITERS = 3
SBYTES = N_COLS * 4
PBYTES = SBYTES + N_COLS                   # packed row: f32 scores' then u8 mask

_CACHE = {}


def build_nc(rows_per_core=ROWS_PER_CORE, n_cols=N_COLS, n_iters=N_ITERS):
    import concourse.bacc as bacc
    import concourse.mybir as mybir
    from concourse.tile import TileContext

    f32 = mybir.dt.float32
    bf16 = mybir.dt.bfloat16
    u8 = mybir.dt.uint8
    Alu = mybir.AluOpType
    Act = mybir.ActivationFunctionType

    nt = rows_per_core // P
    sbytes = n_cols * 4
    pbytes = sbytes + n_cols
    ngrp = 4
    gsz = nt // ngrp
    nc = bacc.Bacc("TRN2", target_bir_lowering=False, debug=False)

    s_h = nc.declare_dram_parameter("s15", [rows_per_core, n_cols], f32, isOutput=False)
    mk_h = nc.declare_dram_parameter("mk", [rows_per_core, n_cols], u8, isOutput=False)
    invk_h = nc.declare_dram_parameter("invk", [P, 8], f32, isOutput=False)
    kvec_h = nc.declare_dram_parameter("kvec", [P, 8], f32, isOutput=False)
    p_h = nc.declare_dram_parameter("p", [rows_per_core, n_cols], f32, isOutput=True)

    s15 = s_h.ap()
    mk = mk_h.ap()
    pout = p_h.ap()

    with TileContext(nc) as tc:
        with (
            tc.tile_pool(name="pm", bufs=nt) as pm,
            tc.tile_pool(name="pmm", bufs=3) as pmm,
            tc.tile_pool(name="pu", bufs=3) as pu,
            tc.tile_pool(name="pv", bufs=1) as pv,
            tc.tile_pool(name="ps1", bufs=1) as ps1,
            tc.tile_pool(name="ps3", bufs=3) as ps3,
        ):
            invk = ps1.tile([P, 8], f32)
            nc.sync.dma_start(out=invk, in_=invk_h.ap())
            kvec = ps1.tile([P, 8], f32)
            nc.sync.dma_start(out=kvec, in_=kvec_h.ap())

            grp = []  # per group: dict(t tiles, a, nega, h_prev, F, d_prev)

            def phase_a(gi):
                """load + mask-fold + top8 + batched warm solve for group gi."""
                tiles = list(range(gi * gsz, (gi + 1) * gsz))
                sh3 = [P, gsz, 8]
                T8 = ps1.tile(sh3, f32, name=f"T8_{gi}", tag=f"T8_{gi}")
                t_tiles = []
                for j, i in enumerate(tiles):
                    t_i = pm.tile([P, n_cols], f32, name=f"t{i}", tag="t")
                    nc.sync.dma_start(out=t_i, in_=s15[i * P:(i + 1) * P, :])
                    m_i = pmm.tile([P, n_cols], u8, name=f"m{i}", tag="m")
                    nc.sync.dma_start(out=m_i, in_=mk[i * P:(i + 1) * P, :])
                    # group 0 folds on DVE (fast start); later groups on GPSIMD
                    eng = nc.vector if gi == 0 else nc.gpsimd
                    eng.tensor_tensor(t_i, t_i, m_i, Alu.mult)
                    nc.vector.max(T8[:, j, :], t_i)
                    t_tiles.append(t_i)

                Mp_b = T8[:, :, 0:1].broadcast_to(sh3)
                invk_b = invk.rearrange("p (o k) -> p o k", o=1).broadcast_to(sh3)
                kvec_b = kvec.rearrange("p (o k) -> p o k", o=1).broadcast_to(sh3)

                z8 = ps1.tile(sh3, f32, name=f"z8_{gi}", tag=f"z8_{gi}")
                nc.vector.tensor_tensor(z8, T8, Mp_b, Alu.subtract)
                q8 = ps1.tile(sh3, f32, name=f"q8_{gi}", tag=f"q8_{gi}")
                nc.vector.tensor_tensor(q8, z8, z8, Alu.mult)

                def cumsum8(src, pref):
                    a1 = ps1.tile(sh3, f32, name=f"{pref}a_{gi}", tag=f"{pref}a_{gi}")
                    nc.vector.tensor_copy(a1[:, :, 0:1], src[:, :, 0:1])
                    nc.vector.tensor_tensor(a1[:, :, 1:8], src[:, :, 1:8], src[:, :, 0:7], Alu.add)
                    a2 = ps1.tile(sh3, f32, name=f"{pref}b_{gi}", tag=f"{pref}b_{gi}")
                    nc.vector.tensor_copy(a2[:, :, 0:2], a1[:, :, 0:2])
                    nc.vector.tensor_tensor(a2[:, :, 2:8], a1[:, :, 2:8], a1[:, :, 0:6], Alu.add)
                    a4 = ps1.tile(sh3, f32, name=f"{pref}c_{gi}", tag=f"{pref}c_{gi}")
                    nc.vector.tensor_copy(a4[:, :, 0:4], a2[:, :, 0:4])
                    nc.vector.tensor_tensor(a4[:, :, 4:8], a2[:, :, 4:8], a2[:, :, 0:4], Alu.add)
                    return a4

                cs = cumsum8(z8, "cs")
                cq = cumsum8(q8, "cq")

                mean = ps1.tile(sh3, f32, name=f"mean_{gi}", tag=f"mean_{gi}")
                nc.vector.tensor_tensor(mean, cs, invk_b, Alu.mult)
                msq = ps1.tile(sh3, f32, name=f"msq_{gi}", tag=f"msq_{gi}")
                nc.vector.tensor_tensor(msq, cq, invk_b, Alu.mult)
                mm = ps1.tile(sh3, f32, name=f"mm_{gi}", tag=f"mm_{gi}")
                nc.vector.tensor_tensor(mm, mean, mean, Alu.mult)
                nc.vector.tensor_tensor(mm, msq, mm, Alu.subtract)
                nc.vector.tensor_tensor(mm, mm, kvec_b, Alu.mult)
                nc.vector.tensor_scalar(mm, mm, -1.0, 1.0, Alu.mult, Alu.add)
                nc.vector.tensor_tensor(mm, mm, invk_b, Alu.mult)
                nc.vector.tensor_scalar(mm, mm, 0.0, None, Alu.max)
                sq = ps1.tile(sh3, f32, name=f"sq_{gi}", tag=f"sq_{gi}")
                nc.scalar.sqrt(sq, mm)
                tauc = ps1.tile(sh3, f32, name=f"tauc_{gi}", tag=f"tauc_{gi}")
                nc.vector.tensor_tensor(tauc, mean, sq, Alu.subtract)

                ind = ps1.tile(sh3, f32, name=f"ind_{gi}", tag=f"ind_{gi}")
                nc.vector.tensor_tensor(ind, tauc, z8, Alu.is_le)
                sel = ps1.tile(sh3, f32, name=f"sel_{gi}", tag=f"sel_{gi}")
                nc.vector.tensor_copy(sel[:, :, 7:8], ind[:, :, 7:8])
                nc.vector.tensor_tensor(sel[:, :, 0:7], ind[:, :, 0:7], ind[:, :, 1:8], Alu.subtract)
                nc.vector.tensor_tensor(tauc, tauc, sel, Alu.mult)

                tau0 = ps1.tile([P, gsz], f32, name=f"tau0_{gi}", tag=f"tau0_{gi}")
                nc.vector.reduce_sum(tau0, tauc, axis=mybir.AxisListType.X)

                a = ps1.tile([P, gsz], f32, name=f"a_{gi}", tag=f"a_{gi}")
                nc.vector.tensor_tensor(a, tau0, T8[:, :, 0], Alu.add)
                nega = ps1.tile([P, gsz], f32, name=f"nega_{gi}", tag=f"nega_{gi}")
                nc.vector.tensor_scalar(nega, a, -1.0, None, Alu.mult)
                grp.append(dict(tiles=tiles, t=t_tiles, a=a, nega=nega,
                                h_prev=None, F=None, d_prev=None))

            def iteration(gi, it):
                g = grp[gi]
                h = ps3.tile([P, gsz], f32, name=f"h{gi}_{it}", tag="h")
                measured = it < 2
                if measured:
                    F = ps3.tile([P, gsz], f32, name=f"F{gi}_{it}", tag="F")
                # DVE-F tiles first so their square+reduce overlaps later relus
                for j in (1, 0):
                    u_j = pu.tile([P, n_cols], f32, name=f"u{gi}_{it}_{j}", tag="u")
                    nc.scalar.activation(
                        u_j, g["t"][j], Act.Relu,
                        bias=g["nega"][:, j:j + 1], scale=1.0,
                        accum_out=h[:, j:j + 1],
                    )
                    if measured:
                        if j == 1:  # DVE path
                            nc.vector.tensor_tensor(u_j, u_j, u_j, Alu.mult)
                            nc.vector.reduce_sum(F[:, j:j + 1], u_j,
                                                 axis=mybir.AxisListType.X)
                        else:            # ACT path
                            v_j = pv.tile([P, n_cols], bf16,
                                          name=f"v{gi}_{it}_{j}", tag="v")
                            nc.scalar.activation(v_j, u_j, Act.Square,
                                                 accum_out=F[:, j:j + 1])
                if not measured:
                    # F = F_prev - (h_prev + h) * d_prev
                    F = ps3.tile([P, gsz], f32, name=f"F{gi}_{it}", tag="F")
                    hs = ps3.tile([P, gsz], f32, name=f"hs{gi}_{it}", tag="hs")
                    nc.vector.tensor_tensor(hs, g["h_prev"], h, Alu.add)
                    nc.vector.tensor_tensor(hs, hs, g["d_prev"], Alu.mult)
                    nc.vector.tensor_tensor(F, g["F"], hs, Alu.subtract)
                # d = (F - 1) / (2 h);  a += d;  nega = -a
                num = ps3.tile([P, gsz], f32, name=f"num{gi}_{it}", tag="num")
                nc.vector.tensor_scalar(num, F, -1.0, None, Alu.add)
                den = ps3.tile([P, gsz], f32, name=f"den{gi}_{it}", tag="den")
                nc.vector.tensor_scalar(den, h, 2.0, None, Alu.mult)
                rd = ps3.tile([P, gsz], f32, name=f"rd{gi}_{it}", tag="rd")
                nc.vector.reciprocal(rd, den)
                nc.vector.tensor_tensor(num, num, rd, Alu.mult)
                nc.vector.tensor_tensor(g["a"], g["a"], num, Alu.add)
                nc.vector.tensor_scalar(g["nega"], g["a"], -1.0, None, Alu.mult)
                g["h_prev"], g["F"], g["d_prev"] = h, F, num

            def final(gi):
                g = grp[gi]
                for j, i in enumerate(g["tiles"]):
                    u_j = pu.tile([P, n_cols], f32, name=f"uf{gi}_{j}", tag="u")
                    if j == 1:  # DVE path
                        nc.vector.tensor_scalar(u_j, g["t"][j], g["a"][:, j:j + 1],
                                                0.0, Alu.subtract, Alu.max)
                        nc.vector.tensor_tensor(u_j, u_j, u_j, Alu.mult)
                        nc.sync.dma_start(out=pout[i * P:(i + 1) * P, :], in_=u_j)
                    else:            # ACT path
                        nc.scalar.activation(u_j, g["t"][j], Act.Relu,
                                             bias=g["nega"][:, j:j + 1], scale=1.0)
                        nc.scalar.activation(u_j, u_j, Act.Square)
                        nc.scalar.dma_start(out=pout[i * P:(i + 1) * P, :], in_=u_j)

            phase_a(0)
            phase_a(1)
            iteration(0, 0)
            phase_a(2)
            iteration(1, 0)
            phase_a(3)
            iteration(0, 1)
            iteration(2, 0)
            iteration(1, 1)
            iteration(3, 0)
            iteration(0, 2)
            iteration(2, 1)
            final(0)
            iteration(1, 2)
            iteration(3, 1)
            final(1)
            iteration(2, 2)
            final(2)
            iteration(3, 2)
            final(3)

    nc.compile()
    return nc


def _host_prep(scores, mask):
    s15 = (np.float32(0.5) * np.asarray(scores, dtype=np.float32) + np.float32(15.0))
    mku8 = np.asarray(mask).astype(np.uint8)
    k = np.arange(1, 9, dtype=np.float32)
    invk = np.tile((np.float32(1.0) / k), (P, 1)).astype(np.float32)
    kvec = np.tile(k, (P, 1)).astype(np.float32)
    return s15, mku8, invk, kvec


def run(scores: np.ndarray, mask: np.ndarray, trace: bool = False, **kw):
    from concourse.bass_utils import run_bass_kernel_spmd

    assert scores.shape == (N_ROWS, N_COLS) and mask.shape == (N_ROWS, N_COLS)
    if "nc" not in _CACHE:
        _CACHE["nc"] = build_nc()
    nc = _CACHE["nc"]

    s15, mku8, invk, kvec = _host_prep(scores, mask)
    rpc = ROWS_PER_CORE
    in_maps = [
        {
            "s15": np.ascontiguousarray(s15[i * rpc:(i + 1) * rpc]),
            "mk": np.ascontiguousarray(mku8[i * rpc:(i + 1) * rpc]),
            "invk": invk,
            "kvec": kvec,
        }
        for i in range(N_CORES)
    ]
    res = run_bass_kernel_spmd(nc, in_maps, list(range(N_CORES)), trace=trace, **kw)
    out = np.concatenate([res.results[i]["p"] for i in range(N_CORES)], axis=0)
    return np.ascontiguousarray(out.astype(np.float32)), res


def kernel(scores: np.ndarray, mask: np.ndarray) -> np.ndarray:
    return run(scores, mask)[0]


if __name__ == "__main__":
    rng = np.random.default_rng(0)
    scores = rng.standard_normal((N_ROWS, N_COLS), dtype=np.float32)
    mask = rng.integers(0, 2, (N_ROWS, N_COLS)).astype(bool)
    out = kernel(scores, mask)
    print("out", out.shape, out.dtype, "rowsum", out.sum(-1)[:4])


# revision 13
# speedup vs baseline: 1.3270x; 1.3270x over previous
"""Trainium2 Bass kernel for entmax-1.5 over rows of a masked [8192, 4096] matrix.

Algorithm (sort-free, validated against the jax reference in float32):
  p_i = relu(z_i - tau)^2 per row, tau s.t. sum_i p_i = 1, z = masked_scores/2.
  Device works in "half-units" t = (0.5*s + 15)*mask (masked -> 0, ~14 below
  any feasible threshold), so p = relu(t - a)^2 with a = rowmax(t) + tau and
  no rescaling anywhere:
    1. top-8 per row (DVE InstMax) -> closed-form entmax threshold of the
       top-8 subset (reference recursion on 8 sorted values, batched across
       tiles, gather-free support selection). Guaranteed lower bound of tau
       (support size is ~21, max 46).
    2. 3 Newton refinements on the full row:  u = relu(t - a) via ACT with
       per-partition bias, accum_out -> h = sum u;  F = sum u^2 measured on
       iters 0,1 (ACT Square+accum / DVE square+reduce, split to balance
       engines) and propagated on iter 2 via the trapezoid identity
       F' = F - (h_prev + h) * delta  (f is piecewise quadratic in tau);
       a += (F - 1) / (2 h).
    3. p = relu(t - a)^2  (ACT Relu;Square on half the tiles, DVE on rest).

Sharding: pure data parallelism — 8192 rows = 1024 rows x 8 cores; per core
8 tiles of [128 partitions x 4096], processed as 2 groups of 4 whose phases
interleave (group 2 loads/masks overlap group 1 compute).

Hardware constraints worked around: pseudo-DMA / ACT / TT instructions accept
very few sync-waits, so scores+mask are packed into one u8 DMA per tile (the
mask-fold waits on a single queue semaphore), packed tiles get dedicated SBUF
slots, and bacc's event-semaphore pass splits remaining multi-waits.
tensor_tensor_reduce crashes this runtime and is not used. Group-2 mask-folds
run on GPSIMD to keep DVE free.

Self-contained: hardcodes scores[8192,4096] f32 + mask[8192,4096] bool.
"""

import sys

import numpy as np

sys.path.insert(0, "/opt/trn_rl_repo")

N_ROWS = 8192
N_COLS = 4096
N_CORES = 8
P = 128
ROWS_PER_CORE = N_ROWS // N_CORES          # 1024
NT = ROWS_PER_CORE // P                    # 8 tiles per core
N_ITERS = 3
SBYTES = N_COLS * 4
PBYTES = SBYTES + N_COLS                   # packed row: f32 scores' then u8 mask

_CACHE = {}


def build_nc(rows_per_core=ROWS_PER_CORE, n_cols=N_COLS, n_iters=N_ITERS):
    import concourse.bacc as bacc
    import concourse.mybir as mybir
    from concourse.tile import TileContext

    f32 = mybir.dt.float32
    bf16 = mybir.dt.bfloat16
    u8 = mybir.dt.uint8
    Alu = mybir.AluOpType
    Act = mybir.ActivationFunctionType

    nt = rows_per_core // P
    sbytes = n_cols * 4
    pbytes = sbytes + n_cols
    ngrp = 4
    gsz = nt // ngrp
    nc = bacc.Bacc("TRN2", target_bir_lowering=False, debug=False)

    s_h = nc.declare_dram_parameter("s15", [rows_per_core, n_cols], f32, isOutput=False)
    mk_h = nc.declare_dram_parameter("mk", [rows_per_core, n_cols], u8, isOutput=False)
    invk_h = nc.declare_dram_parameter("invk", [P, 8], f32, isOutput=False)
    kvec_h = nc.declare_dram_parameter("kvec", [P, 8], f32, isOutput=False)
    p_h = nc.declare_dram_parameter("p", [rows_per_core, n_cols], f32, isOutput=True)

    s15 = s_h.ap()
    mk = mk_h.ap()
    pout = p_h.ap()

    with TileContext(nc) as tc:
        with (
            tc.tile_pool(name="pm", bufs=nt) as pm,
            tc.tile_pool(name="pmm", bufs=3) as pmm,
            tc.tile_pool(name="pu", bufs=3) as pu,
            tc.tile_pool(name="pv", bufs=1) as pv,
            tc.tile_pool(name="ps1", bufs=1) as ps1,
            tc.tile_pool(name="ps3", bufs=3) as ps3,
        ):
            invk = ps1.tile([P, 8], f32)
            nc.sync.dma_start(out=invk, in_=invk_h.ap())
            kvec = ps1.tile([P, 8], f32)
            nc.sync.dma_start(out=kvec, in_=kvec_h.ap())

            grp = []  # per group: dict(t tiles, a, nega, h_prev, F, d_prev)

            def phase_a(gi):
                """load + mask-fold + top8 + batched warm solve for group gi."""
                tiles = list(range(gi * gsz, (gi + 1) * gsz))
                sh3 = [P, gsz, 8]
                T8 = ps1.tile(sh3, f32, name=f"T8_{gi}", tag=f"T8_{gi}")
                t_tiles = []
                for j, i in enumerate(tiles):
                    t_i = pm.tile([P, n_cols], f32, name=f"t{i}", tag="t")
                    nc.sync.dma_start(out=t_i, in_=s15[i * P:(i + 1) * P, :])
                    m_i = pmm.tile([P, n_cols], u8, name=f"m{i}", tag="m")
                    nc.sync.dma_start(out=m_i, in_=mk[i * P:(i + 1) * P, :])
                    nc.vector.tensor_tensor(t_i, t_i, m_i, Alu.mult)
                    nc.vector.max(T8[:, j, :], t_i)
                    t_tiles.append(t_i)

                Mp_b = T8[:, :, 0:1].broadcast_to(sh3)
                invk_b = invk.rearrange("p (o k) -> p o k", o=1).broadcast_to(sh3)
                kvec_b = kvec.rearrange("p (o k) -> p o k", o=1).broadcast_to(sh3)

                z8 = ps1.tile(sh3, f32, name=f"z8_{gi}", tag=f"z8_{gi}")
                nc.vector.tensor_tensor(z8, T8, Mp_b, Alu.subtract)
                q8 = ps1.tile(sh3, f32, name=f"q8_{gi}", tag=f"q8_{gi}")
                nc.vector.tensor_tensor(q8, z8, z8, Alu.mult)

                def cumsum8(src, pref):
                    a1 = ps1.tile(sh3, f32, name=f"{pref}a_{gi}", tag=f"{pref}a_{gi}")
                    nc.vector.tensor_copy(a1[:, :, 0:1], src[:, :, 0:1])
                    nc.vector.tensor_tensor(a1[:, :, 1:8], src[:, :, 1:8], src[:, :, 0:7], Alu.add)
                    a2 = ps1.tile(sh3, f32, name=f"{pref}b_{gi}", tag=f"{pref}b_{gi}")
                    nc.vector.tensor_copy(a2[:, :, 0:2], a1[:, :, 0:2])
                    nc.vector.tensor_tensor(a2[:, :, 2:8], a1[:, :, 2:8], a1[:, :, 0:6], Alu.add)
                    a4 = ps1.tile(sh3, f32, name=f"{pref}c_{gi}", tag=f"{pref}c_{gi}")
                    nc.vector.tensor_copy(a4[:, :, 0:4], a2[:, :, 0:4])
                    nc.vector.tensor_tensor(a4[:, :, 4:8], a2[:, :, 4:8], a2[:, :, 0:4], Alu.add)
                    return a4

                cs = cumsum8(z8, "cs")
                cq = cumsum8(q8, "cq")

                mean = ps1.tile(sh3, f32, name=f"mean_{gi}", tag=f"mean_{gi}")
                nc.vector.tensor_tensor(mean, cs, invk_b, Alu.mult)
                msq = ps1.tile(sh3, f32, name=f"msq_{gi}", tag=f"msq_{gi}")
                nc.vector.tensor_tensor(msq, cq, invk_b, Alu.mult)
                mm = ps1.tile(sh3, f32, name=f"mm_{gi}", tag=f"mm_{gi}")
                nc.vector.tensor_tensor(mm, mean, mean, Alu.mult)
                nc.vector.tensor_tensor(mm, msq, mm, Alu.subtract)
                nc.vector.tensor_tensor(mm, mm, kvec_b, Alu.mult)
                nc.vector.tensor_scalar(mm, mm, -1.0, 1.0, Alu.mult, Alu.add)
                nc.vector.tensor_tensor(mm, mm, invk_b, Alu.mult)
                nc.vector.tensor_scalar(mm, mm, 0.0, None, Alu.max)
                sq = ps1.tile(sh3, f32, name=f"sq_{gi}", tag=f"sq_{gi}")
                nc.scalar.sqrt(sq, mm)
                tauc = ps1.tile(sh3, f32, name=f"tauc_{gi}", tag=f"tauc_{gi}")
                nc.vector.tensor_tensor(tauc, mean, sq, Alu.subtract)

                ind = ps1.tile(sh3, f32, name=f"ind_{gi}", tag=f"ind_{gi}")
                nc.vector.tensor_tensor(ind, tauc, z8, Alu.is_le)
                sel = ps1.tile(sh3, f32, name=f"sel_{gi}", tag=f"sel_{gi}")
                nc.vector.tensor_copy(sel[:, :, 7:8], ind[:, :, 7:8])
                nc.vector.tensor_tensor(sel[:, :, 0:7], ind[:, :, 0:7], ind[:, :, 1:8], Alu.subtract)
                nc.vector.tensor_tensor(tauc, tauc, sel, Alu.mult)

                tau0 = ps1.tile([P, gsz], f32, name=f"tau0_{gi}", tag=f"tau0_{gi}")
                nc.vector.reduce_sum(tau0, tauc, axis=mybir.AxisListType.X)

                a = ps1.tile([P, gsz], f32, name=f"a_{gi}", tag=f"a_{gi}")
                nc.vector.tensor_tensor(a, tau0, T8[:, :, 0], Alu.add)
                nega = ps1.tile([P, gsz], f32, name=f"nega_{gi}", tag=f"nega_{gi}")
                nc.vector.tensor_scalar(nega, a, -1.0, None, Alu.mult)
                grp.append(dict(tiles=tiles, t=t_tiles, a=a, nega=nega,
                                h_prev=None, F=None, d_prev=None))

            def iteration(gi, it):
                g = grp[gi]
                h = ps3.tile([P, gsz], f32, name=f"h{gi}_{it}", tag="h")
                measured = it < 2
                if measured:
                    F = ps3.tile([P, gsz], f32, name=f"F{gi}_{it}", tag="F")
                # DVE-F tiles first so their square+reduce overlaps later relus
                for j in (1, 0):
                    u_j = pu.tile([P, n_cols], f32, name=f"u{gi}_{it}_{j}", tag="u")
                    nc.scalar.activation(
                        u_j, g["t"][j], Act.Relu,
                        bias=g["nega"][:, j:j + 1], scale=1.0,
                        accum_out=h[:, j:j + 1],
                    )
                    if measured:
                        if j == 1:  # DVE path: F = N*(var + mean^2) via bn_stats
                            bns = ps3.tile([P, 8, 6], f32,
                                           name=f"bns{gi}_{it}", tag="bns")
                            for c in range(8):
                                nc.vector.bn_stats(bns[:, c, :],
                                                   u_j[:, c * 512:(c + 1) * 512])
                            mv = ps3.tile([P, 2], f32, name=f"mv{gi}_{it}", tag="mv")
                            nc.vector.bn_aggr(mv, bns.rearrange("p a b -> p (a b)"))
                            m2 = ps3.tile([P, 1], f32, name=f"m2{gi}_{it}", tag="m2")
                            nc.vector.tensor_tensor(m2, mv[:, 0:1], mv[:, 0:1], Alu.mult)
                            nc.vector.tensor_tensor(m2, m2, mv[:, 1:2], Alu.add)
                            nc.vector.tensor_scalar(F[:, j:j + 1], m2, 4096.0, None, Alu.mult)
                        else:            # ACT path
                            v_j = pv.tile([P, n_cols], bf16,
                                          name=f"v{gi}_{it}_{j}", tag="v")
                            nc.scalar.activation(v_j, u_j, Act.Square,
                                                 accum_out=F[:, j:j + 1])
                if not measured:
                    # F = F_prev - (h_prev + h) * d_prev
                    F = ps3.tile([P, gsz], f32, name=f"F{gi}_{it}", tag="F")
                    hs = ps3.tile([P, gsz], f32, name=f"hs{gi}_{it}", tag="hs")
                    nc.vector.tensor_tensor(hs, g["h_prev"], h, Alu.add)
                    nc.vector.tensor_tensor(hs, hs, g["d_prev"], Alu.mult)
                    nc.vector.tensor_tensor(F, g["F"], hs, Alu.subtract)
                # d = (F - 1) / (2 h);  a += d;  nega = -a
                num = ps3.tile([P, gsz], f32, name=f"num{gi}_{it}", tag="num")
                nc.vector.tensor_scalar(num, F, -1.0, None, Alu.add)
                den = ps3.tile([P, gsz], f32, name=f"den{gi}_{it}", tag="den")
                nc.vector.tensor_scalar(den, h, 2.0, None, Alu.mult)
                rd = ps3.tile([P, gsz], f32, name=f"rd{gi}_{it}", tag="rd")
                nc.vector.reciprocal(rd, den)
                nc.vector.tensor_tensor(num, num, rd, Alu.mult)
                nc.vector.tensor_tensor(g["a"], g["a"], num, Alu.add)
                nc.vector.tensor_scalar(g["nega"], g["a"], -1.0, None, Alu.mult)
                g["h_prev"], g["F"], g["d_prev"] = h, F, num

            def final(gi):
                g = grp[gi]
                for j, i in enumerate(g["tiles"]):
                    u_j = pu.tile([P, n_cols], f32, name=f"uf{gi}_{j}", tag="u")
                    if j == 1:  # DVE path
                        nc.vector.tensor_scalar(u_j, g["t"][j], g["a"][:, j:j + 1],
                                                0.0, Alu.subtract, Alu.max)
                        nc.vector.tensor_tensor(u_j, u_j, u_j, Alu.mult)
                        nc.sync.dma_start(out=pout[i * P:(i + 1) * P, :], in_=u_j)
                    else:            # ACT path
                        nc.scalar.activation(u_j, g["t"][j], Act.Relu,
                                             bias=g["nega"][:, j:j + 1], scale=1.0)
                        nc.scalar.activation(u_j, u_j, Act.Square)
                        nc.scalar.dma_start(out=pout[i * P:(i + 1) * P, :], in_=u_j)

            phase_a(0)
            phase_a(1)
            iteration(0, 0)
            phase_a(2)
            iteration(1, 0)
            phase_a(3)
            iteration(0, 1)
            iteration(2, 0)
            iteration(1, 1)
            iteration(3, 0)
            iteration(0, 2)
            iteration(2, 1)
            final(0)
            iteration(1, 2)
            iteration(3, 1)
            final(1)
            iteration(2, 2)
            final(2)
            iteration(3, 2)
            final(3)

    nc.compile()
    return nc


def _host_prep(scores, mask):
    s15 = (np.float32(0.5) * np.asarray(scores, dtype=np.float32) + np.float32(15.0))
    mku8 = np.asarray(mask).astype(np.uint8)
    k = np.arange(1, 9, dtype=np.float32)
    invk = np.tile((np.float32(1.0) / k), (P, 1)).astype(np.float32)
    kvec = np.tile(k, (P, 1)).astype(np.float32)
    return s15, mku8, invk, kvec


def run(scores: np.ndarray, mask: np.ndarray, trace: bool = False, **kw):
    from concourse.bass_utils import run_bass_kernel_spmd

    assert scores.shape == (N_ROWS, N_COLS) and mask.shape == (N_ROWS, N_COLS)
    if "nc" not in _CACHE:
        _CACHE["nc"] = build_nc()
    nc = _CACHE["nc"]

    s15, mku8, invk, kvec = _host_prep(scores, mask)
    rpc = ROWS_PER_CORE
    in_maps = [
        {
            "s15": np.ascontiguousarray(s15[i * rpc:(i + 1) * rpc]),
            "mk": np.ascontiguousarray(mku8[i * rpc:(i + 1) * rpc]),
            "invk": invk,
            "kvec": kvec,
        }
        for i in range(N_CORES)
    ]
    res = run_bass_kernel_spmd(nc, in_maps, list(range(N_CORES)), trace=trace, **kw)
    out = np.concatenate([res.results[i]["p"] for i in range(N_CORES)], axis=0)
    return np.ascontiguousarray(out.astype(np.float32)), res


def kernel(scores: np.ndarray, mask: np.ndarray) -> np.ndarray:
    return run(scores, mask)[0]


if __name__ == "__main__":
    rng = np.random.default_rng(0)
    scores = rng.standard_normal((N_ROWS, N_COLS), dtype=np.float32)
    mask = rng.integers(0, 2, (N_ROWS, N_COLS)).astype(bool)
    out = kernel(scores, mask)
    print("out", out.shape, out.dtype, "rowsum", out.sum(-1)[:4])
